# revision 18
# baseline (speedup 1.0000x reference)
"""ALiBi causal multi-head attention on 8 TRN2 NeuronCores.

Problem: x[2,2048,1024] -> qkv proj (16 heads, d=64) -> ALiBi-biased causal
softmax attention -> out proj [1024,1024] + bias.

Sharding: core = (batch b in {0,1}) x (head-group g in {0..3}, 4 heads each).
Each core computes its batch's QKV for its 4 heads, full causal attention,
and a partial output projection (its heads' rows of w_out). Host sums the 4
head-group partials per batch and adds b_out.

On-chip layout tricks:
  - x is pre-transposed AND column-blocked on host (xtb [4*1024, 512] fp16),
    so the contraction dim is on partitions for all projection matmuls with
    no on-chip transposes, and the DMA streams chunk-by-chunk: chunk-0
    projections start ~9us in instead of waiting for the full 4MB.
  - w_q (pre-scaled), w_k, w_v ride in one packed wqkv tensor: 8 DMAs of
    [128, 768] instead of 24 small ones (each DMA costs ~0.5us fixed).
  - sim is computed transposed (simT [keys, queries]) so exp(simT) is
    directly the lhsT of the attention*V matmul -- no P transposes.
  - The ALiBi bias slope*(j-i) is folded into the sim matmul via two extra
    contraction rows: kT_aug = [K^T; j; 1], qT_aug = [Q^T*scale; slope;
    -slope*i].
  - The causal mask is ALSO a matmul: -30000*max(0, p-f) =
    sum_r U[r,p]*R[r,f] with U[r,p]=[r<=p], R[r,f]=-30000*[r>f], so the
    strict upper triangle of each diagonal 128x128 block gets <= -30000
    accumulated into sim PSUM by one extra 128-col matmul -- no vector-
    engine pass, no cross-engine hop in the softmax chain.
  - The softmax denominator comes out of the attention*V matmul by
    appending a ones column to V (V_aug [keys, 65]); row 64 of the
    transposed output is sum_j P[j, i].
  - Heads are processed in pairs writing one [128, 1024] PSUM sim tile
    (two banks); a single ACTIVATE exp's both heads' tiles, halving the
    Scalar engine's fixed per-instruction costs in the exp-bound endgame.
  - AV accumulators are copied off PSUM right after their last matmul so
    the two psav banks recycle ~1.5us earlier; softmax normalization then
    runs entirely from SBUF (reciprocal on DVE, broadcast + multiply on the
    otherwise-idle Pool engine).
  - qT/kT are zero-padded to 96 partitions (K<=64 matmuls run at HALF rate
    on TRN2); pad zeroing via fp32-bitcast memsets split DVE/Pool at the
    head.
  - DMA queues: Sync carries the bulk; Activation carries block halves +
    small transfers + half the outputs.  Keeping bulk DMA off the Scalar
    engine matters: an engine stalls once its 16-deep DGE ring fills.
"""

import sys

for _p in ("/opt/trn_rl_repo", "/root/.axon_site/_ro/trn_rl_repo"):
    if _p not in sys.path:
        sys.path.append(_p)

import numpy as np
from math import log2, floor

import concourse.bass as bass
import concourse.mybir as mybir
import concourse.tile as tile
from concourse import bacc, bass_utils

F32 = mybir.dt.float32
F16 = mybir.dt.float16
AF = mybir.ActivationFunctionType

B = 2          # batches
NH = 16        # total heads
H = 4          # heads per core
D = 64         # head dim
N = 2048       # sequence length
DM = 1024      # model dim
CH = 512       # query chunk (free dim of sim/av matmuls)
NCH = N // CH  # 4
KT = N // 128  # 16 key tiles
KD = DM // 128 # 8 contraction tiles for projections
SCALE = D ** -0.5
MASK_NEG = -30000.0
N_WARMUP = 17  # dummy matmuls to warm the PE clock during the DMA head


def _slopes(heads):
    def pow2_slopes(n):
        start = 2 ** (-(2 ** (-(log2(n) - 3))))
        return [start * (start ** i) for i in range(n)]
    if log2(heads).is_integer():
        return pow2_slopes(heads)
    c = 2 ** floor(log2(heads))
    return pow2_slopes(c) + pow2_slopes(2 * c)[0::2][: heads - c]


def build_program():
    nc = bacc.Bacc("TRN2", target_bir_lowering=False, debug=False, num_devices=8)
    xtb = nc.dram_tensor("xtb", [NCH * DM, CH], F16, kind="ExternalInput").ap()
    # packed [wq | wk | wv], each H*D columns
    wqkv = nc.dram_tensor("wqkv", [DM, 3 * H * D], F16,
                          kind="ExternalInput").ap()
    wo = nc.dram_tensor("wo", [H * D, DM], F16, kind="ExternalInput").ap()
    qaug = nc.dram_tensor("qaug", [H, 2, N], F16, kind="ExternalInput").ap()
    kaug = nc.dram_tensor("kaug", [2, N], F16, kind="ExternalInput").ap()
    # causal-mask factor matrices: U^T R = -30000 * max(0, p - f)
    um = nc.dram_tensor("um", [128, 128], F16, kind="ExternalInput").ap()
    rm = nc.dram_tensor("rm", [128, 128], F16, kind="ExternalInput").ap()
    out = nc.dram_tensor("out", [N, DM], F16, kind="ExternalOutput").ap()

    QO, KO, VO = 0, H * D, 2 * H * D  # column offsets in packed weights

    with tile.TileContext(nc) as tc:
        with tc.tile_pool(name="persist", bufs=1) as cp:
            # ---- persistent tiles ----
            um_sb = cp.tile([128, 128], F16, tag="um", name="um_sb")
            rm_sb = cp.tile([128, 128], F16, tag="rm", name="rm_sb")
            xt = [cp.tile([128, N], F16, tag=f"xt{k}", name=f"xt{k}")
                  for k in range(KD)]
            wt = [cp.tile([128, 3 * H * D], F16, tag=f"w{k}", name=f"w{k}")
                  for k in range(KD)]
            qt = [cp.tile([96, N], F16, tag=f"qt{h}", name=f"qt{h}")
                  for h in range(H)]
            kt = [cp.tile([96, N], F16, tag=f"kt{h}", name=f"kt{h}")
                  for h in range(H)]
            wot = [cp.tile([128, DM], F16, tag=f"wo{k}", name=f"wo{k}")
                   for k in range(2)]

            # zero the 64:96 pad rows; the aug DMA overwrites rows 64:66
            # afterwards.  DVE zeroes qt, Pool zeroes kt -- both idle in the
            # head window.
            for h in range(H):
                nc.vector.memset(qt[h][64:96, :], 0.0)
                nc.gpsimd.memset(kt[h][64:96, :], 0.0)

            # --- input DMAs.  Bulk on qSP (Sync absorbs ring-full stalls);
            # qAct gets block halves and the small transfers. ---
            def dma_block(b, eng_k=None):
                for k in range(KD):
                    eng = nc.sync if (eng_k is None or k < eng_k) \
                        else nc.scalar
                    eng.dma_start(
                        xt[k][:, CH * b:CH * (b + 1)],
                        xtb[DM * b + 128 * k:DM * b + 128 * (k + 1), :])

            # qAct: b0 second half, masks, augs, b1/b2 second halves
            for k in range(KD // 2, KD):
                nc.scalar.dma_start(
                    xt[k][:, 0:CH], xtb[128 * k:128 * (k + 1), :])
            nc.scalar.dma_start(um_sb[:], um[:])
            nc.scalar.dma_start(rm_sb[:], rm[:])
            for h in range(H):
                nc.scalar.dma_start(qt[h][64:66, :], qaug[h])
                nc.scalar.dma_start(kt[h][64:66, :], kaug[:])
            # qSP: weights, b0 first half, wo, then block splits
            for k in range(KD):
                nc.sync.dma_start(wt[k][:], wqkv[128 * k:128 * (k + 1), :])
            for k in range(KD // 2):
                nc.sync.dma_start(
                    xt[k][:, 0:CH], xtb[128 * k:128 * (k + 1), :])
            for k in range(2):
                nc.sync.dma_start(wot[k][:], wo[128 * k:128 * (k + 1), :])
            for k in range(KD):
                (nc.sync if k < KD // 2 else nc.scalar).dma_start(
                    xt[k][:, CH:2 * CH],
                    xtb[DM + 128 * k:DM + 128 * (k + 1), :])
            for k in range(KD):
                (nc.sync if k < KD // 2 else nc.scalar).dma_start(
                    xt[k][:, 2 * CH:3 * CH],
                    xtb[2 * DM + 128 * k:2 * DM + 128 * (k + 1), :])
            dma_block(3)

            vsb = []
            for r in range(KT):
                t = cp.tile([128, 65 * H], F16, tag=f"v{r}", name=f"v{r}")
                for h in range(H):
                    nc.gpsimd.memset(t[:, 65 * h + 64:65 * h + 65], 1.0)
                vsb.append(t)

            avt = []
            for p in range(2):
                t = cp.tile([128, N], F16, tag=f"avt{p}", name=f"avt{p}")
                avt.append(t)

            # scratch operand for PE warm-up (contents irrelevant)
            warm = cp.tile([128, CH], F16, tag="warm", name="warm")
            nc.vector.memset(warm[:], 0.0)

            with tc.tile_pool(name="pvo", bufs=2, space="PSUM") as pvo, \
                 tc.tile_pool(name="pssim", bufs=2, space="PSUM") as pss, \
                 tc.tile_pool(name="psav", bufs=2, space="PSUM") as psa, \
                 tc.tile_pool(name="ptp", bufs=4) as ptp, \
                 tc.tile_pool(name="avsb", bufs=4) as avs, \
                 tc.tile_pool(name="smsb", bufs=6) as smsb, \
                 tc.tile_pool(name="osb", bufs=3) as osb:
                # PE warm-up: garbage matmuls with no input deps ramp the PE
                # clock while the weights + first xtb block land.
                ps_w = pvo.tile([128, CH], F32, tag="pvo", name="ps_warm")
                for i in range(N_WARMUP):
                    nc.tensor.matmul(ps_w[:], warm[:, 0:128], warm[:],
                                     start=True, stop=True)

                def vproj_rows(c):
                    # V rows 4c..4c+3 (keys of chunk c); one strided DVE copy
                    # moves all four heads' 64-col slices into the 65-stride
                    # vsb layout (ones column at offset 64 is preset).
                    for r in range(4 * c, 4 * c + 4):
                        ps = pvo.tile([128, CH], F32, tag="pvo",
                                      name=f"psv{r}")
                        for k in range(KD):
                            nc.tensor.matmul(
                                ps[:, 0:H * D],
                                xt[k][:, 128 * r:128 * (r + 1)],
                                wt[k][:, VO:VO + H * D],
                                start=(k == 0), stop=(k == KD - 1))
                        dst = vsb[r][:, 0:65 * H].rearrange(
                            "p (h w) -> p h w", h=H)[:, :, 0:64]
                        src = ps[:, 0:H * D].rearrange(
                            "p (h w) -> p h w", h=H)
                        nc.vector.tensor_copy(dst, src)

                def qkproj(c):
                    # qT/kT columns for query chunk c.  Chunk 0's copies run
                    # in the head window where Scalar is still idle.
                    ceng = nc.scalar if c == 0 else nc.vector
                    for hp in range(H // 2):
                        for base, dst in ((QO, qt), (KO, kt)):
                            ps = pvo.tile([128, CH], F32, tag="pvo",
                                          name=f"psqk{hp}_{c}")
                            for k in range(KD):
                                nc.tensor.matmul(
                                    ps[:],
                                    wt[k][:, base + 128 * hp:
                                          base + 128 * (hp + 1)],
                                    xt[k][:, CH * c:CH * (c + 1)],
                                    start=(k == 0), stop=(k == KD - 1))
                            for j in range(2):
                                d = dst[2 * hp + j][0:64,
                                                    CH * c:CH * (c + 1)]
                                s = ps[64 * j:64 * (j + 1), :]
                                if ceng is nc.scalar:
                                    ceng.activation(d, s, AF.Copy)
                                else:
                                    ceng.tensor_copy(d, s)

                def normalize(c, h, av_sb, dn, halves):
                    # avt = av_sb[0:64] / denom, all SBUF-side: reciprocal on
                    # DVE (from the partition-0 staged denom row), broadcast
                    # on Pool, multiply on DVE.
                    nh = 2 if halves else 1
                    w = CH // nh
                    for i in range(nh):
                        sl = slice(w * i, w * (i + 1))
                        rc = smsb.tile([1, w], F32, tag="rc",
                                       name=f"rc{h}_{c}_{i}")
                        nc.vector.reciprocal_approx_fast(rc[:], dn[0:1, sl])
                        rcb = smsb.tile([D, w], F32, tag="rcb",
                                        name=f"rcb{h}_{c}_{i}")
                        nc.gpsimd.partition_broadcast(rcb[:], rc[:])
                        nc.vector.tensor_mul(
                            avt[h // 2][64 * (h % 2):64 * (h % 2) + 64,
                                        CH * c + w * i:CH * c + w * (i + 1)],
                            av_sb[0:64, sl], rcb[:])

                def attn_pair(c, hp):
                    # heads (2hp, 2hp+1) share [128, 1024] sim tiles and one
                    # exp ACTIVATE per key tile
                    h0, h1 = 2 * hp, 2 * hp + 1
                    nkt = 4 * c + 4
                    av0 = psa.tile([65, CH], F32, tag="psav",
                                   name=f"psav{h0}_{c}")
                    av1 = psa.tile([65, CH], F32, tag="psav",
                                   name=f"psav{h1}_{c}")
                    for t in range(nkt):
                        s = t - 4 * c
                        lo = 128 * s if s >= 0 else 0
                        ps2 = pss.tile([128, 2 * CH], F32, tag="pssim",
                                       name=f"pssim{hp}_{c}_{t}")
                        for j, h in ((0, h0), (1, h1)):
                            o = CH * j
                            nc.tensor.matmul(
                                ps2[:, o + lo:o + CH],
                                kt[h][0:96, 128 * t:128 * (t + 1)],
                                qt[h][0:96, CH * c + lo:CH * (c + 1)],
                                start=True, stop=(s < 0),
                                skip_group_check=True)
                            if s >= 0:
                                # causal mask via constant-factor matmul
                                nc.tensor.matmul(
                                    ps2[:, o + lo:o + lo + 128],
                                    um_sb[:], rm_sb[:],
                                    start=False, stop=True,
                                    skip_group_check=True)
                        pt2 = ptp.tile([128, 2 * CH], F16, tag="pt",
                                       name=f"pt{hp}_{c}_{t}")
                        if s >= 0:
                            ia = ps2[:, 0:2 * CH].rearrange(
                                "p (two f) -> p two f", two=2)[:, :, lo:CH]
                            oa = pt2[:, 0:2 * CH].rearrange(
                                "p (two f) -> p two f", two=2)[:, :, lo:CH]
                            nc.scalar.activation(oa, ia, AF.Exp)
                        else:
                            nc.scalar.activation(pt2[:], ps2[:], AF.Exp)
                        for av, h, j in ((av0, h0, 0), (av1, h1, 1)):
                            nc.tensor.matmul(
                                av[:, lo:CH],
                                vsb[t][:, 65 * h:65 * h + 65],
                                pt2[:, CH * j + lo:CH * (j + 1)],
                                start=(t == 0), stop=(t == nkt - 1))
                    # drain the AV accumulators to SBUF so the psav banks
                    # recycle immediately; normalize runs from SBUF
                    halves = (c == NCH - 1)
                    for av, h in ((av0, h0), (av1, h1)):
                        dn = smsb.tile([1, CH], F32, tag="dn",
                                       name=f"dn{h}_{c}")
                        nc.vector.tensor_copy(dn[:], av[64:65, :])
                        av_sb = avs.tile([64, CH], F32, tag="avsb",
                                         name=f"avsb{h}_{c}")
                        nc.vector.tensor_copy(av_sb[:], av[0:64, :])
                        normalize(c, h, av_sb, dn, halves)

                def outproj(c):
                    for u in range(4 * c, 4 * c + 4):
                        o_sb = osb.tile([128, DM], F16, tag="osb",
                                        name=f"osb{u}")
                        for nchk in range(2):
                            ps = pvo.tile([128, CH], F32, tag="pvo",
                                          name=f"psout{u}_{nchk}")
                            for kk in range(2):
                                nc.tensor.matmul(
                                    ps[:],
                                    avt[kk][:, 128 * u:128 * (u + 1)],
                                    wot[kk][:, CH * nchk:CH * (nchk + 1)],
                                    start=(kk == 0), stop=(kk == 1))
                            # last chunk: scalar's exps are done, alternate
                            # the drain copies between Scalar and DVE
                            if c == NCH - 1 and (2 * u + nchk) % 2 == 1:
                                nc.scalar.activation(
                                    o_sb[:, CH * nchk:CH * (nchk + 1)],
                                    ps[:], AF.Copy)
                            else:
                                nc.vector.tensor_copy(
                                    o_sb[:, CH * nchk:CH * (nchk + 1)], ps[:])
                        # output DMAs split across both queues by parity so
                        # the final 1MB drains in parallel
                        deng = nc.sync if u % 2 == 0 else nc.scalar
                        deng.dma_start(out[128 * u:128 * (u + 1), :],
                                       o_sb[:])

                vproj_rows(0)
                qkproj(0)
                for c in range(NCH):
                    attn_pair(c, 0)
                    if c + 1 < NCH:
                        vproj_rows(c + 1)
                    attn_pair(c, 1)
                    if c + 1 < NCH:
                        qkproj(c + 1)
                    outproj(c)

    nc.compile()
    return nc


def make_in_maps(x, w_qkv, w_out):
    """Per-core numpy input dicts. Core c = batch (c // 4) x head-group (c % 4)."""
    slopes = _slopes(NH)
    pos = np.arange(N, dtype=np.float32)
    kaug = np.stack([pos.astype(np.float16), np.ones(N, np.float16)])
    idx = np.arange(128)
    um = (idx[:, None] <= idx[None, :]).astype(np.float16)         # U[r,p]
    rmm = np.where(idx[:, None] > idx[None, :],
                   np.float16(MASK_NEG), np.float16(0.0))          # R[r,f]

    # column-blocked transposed x: [NCH*DM, CH], block b = xT[:, CH*b:CH*(b+1)]
    xtb16 = []
    for b in range(B):
        xT = np.ascontiguousarray(x[b].T).astype(np.float16)  # [DM, N]
        xtb16.append(np.ascontiguousarray(
            xT.reshape(DM, NCH, CH).transpose(1, 0, 2).reshape(NCH * DM, CH)))

    in_maps = []
    for c in range(8):
        b, g = c // 4, c % 4
        hg0 = H * g
        cols = slice(hg0 * D, (hg0 + H) * D)
        wqkv = np.concatenate([
            (w_qkv[:, cols.start:cols.stop] * SCALE).astype(np.float16),
            w_qkv[:, DM + cols.start:DM + cols.stop].astype(np.float16),
            w_qkv[:, 2 * DM + cols.start:2 * DM + cols.stop].astype(np.float16),
        ], axis=1)
        wo = w_out[cols, :].astype(np.float16)
        qa = np.empty((H, 2, N), np.float16)
        for h in range(H):
            s16 = np.float16(slopes[hg0 + h])
            qa[h, 0, :] = s16
            qa[h, 1, :] = (-np.float32(s16) * pos).astype(np.float16)
        in_maps.append({
            "xtb": xtb16[b], "wqkv": wqkv, "wo": wo,
            "qaug": qa, "kaug": kaug, "um": um, "rm": rmm,
        })
    return in_maps


_NC_CACHE = []


def _get_nc():
    if not _NC_CACHE:
        _NC_CACHE.append(build_program())
    return _NC_CACHE[0]


def run_cores(in_maps, **kw):
    nc = _get_nc()
    return bass_utils.run_bass_kernel_spmd(nc, in_maps, core_ids=list(range(8)), **kw)


def kernel(x, w_qkv, w_out, b_out):
    x = np.asarray(x, np.float32)
    w_qkv = np.asarray(w_qkv, np.float32)
    w_out = np.asarray(w_out, np.float32)
    b_out = np.asarray(b_out, np.float32)
    res = run_cores(make_in_maps(x, w_qkv, w_out))
    out = np.zeros((B, N, DM), np.float32)
    for c in range(8):
        out[c // 4] += res.results[c]["out"].astype(np.float32)
    out += b_out[None, None, :]
    return out


# revision 22
# speedup vs baseline: 1.0188x; 1.0188x over previous
"""ALiBi causal multi-head attention on 8 TRN2 NeuronCores.

Problem: x[2,2048,1024] -> qkv proj (16 heads, d=64) -> ALiBi-biased causal
softmax attention -> out proj [1024,1024] + bias.

Sharding: core = (batch b in {0,1}) x (head-group g in {0..3}, 4 heads each).
Each core computes its batch's QKV for its 4 heads, full causal attention,
and a partial output projection (its heads' rows of w_out). Host sums the 4
head-group partials per batch and adds b_out.

On-chip layout tricks:
  - x is pre-transposed AND column-blocked on host (xtb [4*1024, 512] fp16),
    so the contraction dim is on partitions for all projection matmuls with
    no on-chip transposes, and the DMA streams chunk-by-chunk: chunk-0
    projections start ~9us in instead of waiting for the full 4MB.
  - w_q (pre-scaled), w_k, w_v ride in one packed wqkv tensor: 8 DMAs of
    [128, 768] instead of 24 small ones (each DMA costs ~0.5us fixed).
  - sim is computed transposed (simT [keys, queries]) so exp(simT) is
    directly the lhsT of the attention*V matmul -- no P transposes.
  - The ALiBi bias slope*(j-i) is folded into the sim matmul via two extra
    contraction rows: kT_aug = [K^T; j; 1], qT_aug = [Q^T*scale; slope;
    -slope*i].
  - The causal mask is ALSO a matmul: -30000*max(0, p-f) =
    sum_r U[r,p]*R[r,f] with U[r,p]=[r<=p], R[r,f]=-30000*[r>f], so the
    strict upper triangle of each diagonal 128x128 block gets <= -30000
    accumulated into sim PSUM by one extra 128-col matmul -- no vector-
    engine pass, no cross-engine hop in the softmax chain.
  - The softmax denominator comes out of the attention*V matmul by
    appending a ones column to V (V_aug [keys, 65]); row 64 of the
    transposed output is sum_j P[j, i].
  - Heads are processed in pairs writing one [128, 1024] PSUM sim tile
    (two banks); a single ACTIVATE exp's both heads' tiles, halving the
    Scalar engine's fixed per-instruction costs in the exp-bound endgame.
  - AV accumulators are copied off PSUM right after their last matmul so
    the two psav banks recycle ~1.5us earlier; softmax normalization then
    runs entirely from SBUF (reciprocal on DVE, broadcast + multiply on the
    otherwise-idle Pool engine).
  - qT/kT are zero-padded to 96 partitions (K<=64 matmuls run at HALF rate
    on TRN2); pad zeroing via fp32-bitcast memsets split DVE/Pool at the
    head.
  - DMA queues: Sync carries the bulk; Activation carries block halves +
    small transfers + half the outputs.  Keeping bulk DMA off the Scalar
    engine matters: an engine stalls once its 16-deep DGE ring fills.
"""

import sys

for _p in ("/opt/trn_rl_repo", "/root/.axon_site/_ro/trn_rl_repo"):
    if _p not in sys.path:
        sys.path.append(_p)

import numpy as np
from math import log2, floor

import concourse.bass as bass
import concourse.mybir as mybir
import concourse.tile as tile
from concourse import bacc, bass_utils

F32 = mybir.dt.float32
F16 = mybir.dt.float16
AF = mybir.ActivationFunctionType

B = 2          # batches
NH = 16        # total heads
H = 4          # heads per core
D = 64         # head dim
N = 2048       # sequence length
DM = 1024      # model dim
CH = 512       # query chunk (free dim of sim/av matmuls)
NCH = N // CH  # 4
KT = N // 128  # 16 key tiles
KD = DM // 128 # 8 contraction tiles for projections
SCALE = D ** -0.5
MASK_NEG = -30000.0
N_WARMUP = 17  # dummy matmuls to warm the PE clock during the DMA head


def _slopes(heads):
    def pow2_slopes(n):
        start = 2 ** (-(2 ** (-(log2(n) - 3))))
        return [start * (start ** i) for i in range(n)]
    if log2(heads).is_integer():
        return pow2_slopes(heads)
    c = 2 ** floor(log2(heads))
    return pow2_slopes(c) + pow2_slopes(2 * c)[0::2][: heads - c]


def build_program():
    nc = bacc.Bacc("TRN2", target_bir_lowering=False, debug=False, num_devices=8)
    xtb = nc.dram_tensor("xtb", [NCH * DM, CH], F16, kind="ExternalInput").ap()
    # packed [wq | wk | wv], each H*D columns
    wqkv = nc.dram_tensor("wqkv", [DM, 3 * H * D], F16,
                          kind="ExternalInput").ap()
    wo = nc.dram_tensor("wo", [H * D, DM], F16, kind="ExternalInput").ap()
    qaug = nc.dram_tensor("qaug", [H, 2, N], F16, kind="ExternalInput").ap()
    kaug = nc.dram_tensor("kaug", [2, N], F16, kind="ExternalInput").ap()
    # causal-mask factor matrices: U^T R = -30000 * max(0, p - f)
    um = nc.dram_tensor("um", [128, 128], F16, kind="ExternalInput").ap()
    rm = nc.dram_tensor("rm", [128, 128], F16, kind="ExternalInput").ap()
    out = nc.dram_tensor("out", [N, DM], F16, kind="ExternalOutput").ap()

    QO, KO, VO = 0, H * D, 2 * H * D  # column offsets in packed weights

    with tile.TileContext(nc) as tc:
        with tc.tile_pool(name="persist", bufs=1) as cp:
            # ---- persistent tiles ----
            um_sb = cp.tile([128, 128], F16, tag="um", name="um_sb")
            rm_sb = cp.tile([128, 128], F16, tag="rm", name="rm_sb")
            xt = [cp.tile([128, N], F16, tag=f"xt{k}", name=f"xt{k}")
                  for k in range(KD)]
            wt = [cp.tile([128, 3 * H * D], F16, tag=f"w{k}", name=f"w{k}")
                  for k in range(KD)]
            qt = [cp.tile([96, N], F16, tag=f"qt{h}", name=f"qt{h}")
                  for h in range(H)]
            kt = [cp.tile([96, N], F16, tag=f"kt{h}", name=f"kt{h}")
                  for h in range(H)]
            wot = [cp.tile([128, DM], F16, tag=f"wo{k}", name=f"wo{k}")
                   for k in range(2)]

            # scratch operand for PE warm-up (contents irrelevant); memset
            # first so the warm-up matmuls start at ~1us
            warm = cp.tile([128, CH], F16, tag="warm", name="warm")
            nc.vector.memset(warm[:], 0.0)

            # zero the 64:96 pad rows; the aug DMA overwrites rows 64:66
            # afterwards.  DVE zeroes qt, Pool zeroes kt -- both idle in the
            # head window.
            for h in range(H):
                nc.vector.memset(qt[h][64:96, :], 0.0)
                nc.gpsimd.memset(kt[h][64:96, :], 0.0)

            # --- input DMAs.  qSP: weights + late blocks + all outputs
            # (Sync absorbs ring-full stalls; nothing else runs there).
            # qAct: whole block 0 + the small transfers + b1/b2 halves;
            # the Scalar engine only issues these 26 triggers up front and
            # is free well before its first exp. ---
            def blk(b, k):
                return (xt[k][:, CH * b:CH * (b + 1)],
                        xtb[DM * b + 128 * k:DM * b + 128 * (k + 1), :])

            for k in range(KD):
                nc.scalar.dma_start(*blk(0, k))
            nc.scalar.dma_start(um_sb[:], um[:])
            nc.scalar.dma_start(rm_sb[:], rm[:])
            for h in range(H):
                nc.scalar.dma_start(qt[h][64:66, :], qaug[h])
                nc.scalar.dma_start(kt[h][64:66, :], kaug[:])
            for k in range(KD):
                nc.sync.dma_start(wt[k][:], wqkv[128 * k:128 * (k + 1), :])
            for k in range(KD // 2):
                nc.sync.dma_start(*blk(1, k))
            for k in range(2):
                nc.sync.dma_start(wot[k][:], wo[128 * k:128 * (k + 1), :])
            for k in range(KD // 2, KD):
                nc.scalar.dma_start(*blk(1, k))
            for k in range(KD // 2):
                nc.sync.dma_start(*blk(2, k))
            for k in range(KD // 2, KD):
                nc.scalar.dma_start(*blk(2, k))
            for k in range(KD):
                nc.sync.dma_start(*blk(3, k))

            vsb = []
            for r in range(KT):
                t = cp.tile([128, 65 * H], F16, tag=f"v{r}", name=f"v{r}")
                for h in range(H):
                    nc.gpsimd.memset(t[:, 65 * h + 64:65 * h + 65], 1.0)
                vsb.append(t)

            avt = []
            for p in range(2):
                t = cp.tile([128, N], F16, tag=f"avt{p}", name=f"avt{p}")
                avt.append(t)

            with tc.tile_pool(name="pvo", bufs=2, space="PSUM") as pvo, \
                 tc.tile_pool(name="pssim", bufs=2, space="PSUM") as pss, \
                 tc.tile_pool(name="psav", bufs=2, space="PSUM") as psa, \
                 tc.tile_pool(name="ptp", bufs=4) as ptp, \
                 tc.tile_pool(name="avsb", bufs=4) as avs, \
                 tc.tile_pool(name="smsb", bufs=6) as smsb, \
                 tc.tile_pool(name="osb", bufs=3) as osb:
                # PE warm-up: garbage matmuls with no input deps ramp the PE
                # clock while the weights + first xtb block land.
                ps_w = pvo.tile([128, CH], F32, tag="pvo", name="ps_warm")
                for i in range(N_WARMUP):
                    nc.tensor.matmul(ps_w[:], warm[:, 0:128], warm[:],
                                     start=True, stop=True)

                def vproj_rows(c):
                    # V rows 4c..4c+3 (keys of chunk c); one strided DVE copy
                    # moves all four heads' 64-col slices into the 65-stride
                    # vsb layout (ones column at offset 64 is preset).
                    for r in range(4 * c, 4 * c + 4):
                        ps = pvo.tile([128, CH], F32, tag="pvo",
                                      name=f"psv{r}")
                        for k in range(KD):
                            nc.tensor.matmul(
                                ps[:, 0:H * D],
                                xt[k][:, 128 * r:128 * (r + 1)],
                                wt[k][:, VO:VO + H * D],
                                start=(k == 0), stop=(k == KD - 1))
                        dst = vsb[r][:, 0:65 * H].rearrange(
                            "p (h w) -> p h w", h=H)[:, :, 0:64]
                        src = ps[:, 0:H * D].rearrange(
                            "p (h w) -> p h w", h=H)
                        nc.vector.tensor_copy(dst, src)

                def qkproj(c):
                    # qT/kT columns for query chunk c
                    for hp in range(H // 2):
                        for base, dst in ((QO, qt), (KO, kt)):
                            ps = pvo.tile([128, CH], F32, tag="pvo",
                                          name=f"psqk{hp}_{c}")
                            for k in range(KD):
                                nc.tensor.matmul(
                                    ps[:],
                                    wt[k][:, base + 128 * hp:
                                          base + 128 * (hp + 1)],
                                    xt[k][:, CH * c:CH * (c + 1)],
                                    start=(k == 0), stop=(k == KD - 1))
                            for j in range(2):
                                nc.vector.tensor_copy(
                                    dst[2 * hp + j][0:64,
                                                    CH * c:CH * (c + 1)],
                                    ps[64 * j:64 * (j + 1), :])

                def normalize(c, h, av_sb, dn, halves):
                    # avt = av_sb[0:64] / denom, all SBUF-side: reciprocal on
                    # DVE (from the partition-0 staged denom row), broadcast
                    # on Pool, multiply on DVE.
                    nh = 2 if halves else 1
                    w = CH // nh
                    for i in range(nh):
                        sl = slice(w * i, w * (i + 1))
                        rc = smsb.tile([1, w], F32, tag="rc",
                                       name=f"rc{h}_{c}_{i}")
                        nc.vector.reciprocal_approx_fast(rc[:], dn[0:1, sl])
                        rcb = smsb.tile([D, w], F32, tag="rcb",
                                        name=f"rcb{h}_{c}_{i}")
                        nc.gpsimd.partition_broadcast(rcb[:], rc[:])
                        nc.vector.tensor_mul(
                            avt[h // 2][64 * (h % 2):64 * (h % 2) + 64,
                                        CH * c + w * i:CH * c + w * (i + 1)],
                            av_sb[0:64, sl], rcb[:])

                def attn_pair(c, hp):
                    # heads (2hp, 2hp+1) share [128, 1024] sim tiles and one
                    # exp ACTIVATE per key tile
                    h0, h1 = 2 * hp, 2 * hp + 1
                    nkt = 4 * c + 4
                    av0 = psa.tile([65, CH], F32, tag="psav",
                                   name=f"psav{h0}_{c}")
                    av1 = psa.tile([65, CH], F32, tag="psav",
                                   name=f"psav{h1}_{c}")
                    for t in range(nkt):
                        s = t - 4 * c
                        lo = 128 * s if s >= 0 else 0
                        ps2 = pss.tile([128, 2 * CH], F32, tag="pssim",
                                       name=f"pssim{hp}_{c}_{t}")
                        for j, h in ((0, h0), (1, h1)):
                            o = CH * j
                            nc.tensor.matmul(
                                ps2[:, o + lo:o + CH],
                                kt[h][0:96, 128 * t:128 * (t + 1)],
                                qt[h][0:96, CH * c + lo:CH * (c + 1)],
                                start=True, stop=(s < 0),
                                skip_group_check=True)
                            if s >= 0:
                                # causal mask via constant-factor matmul
                                nc.tensor.matmul(
                                    ps2[:, o + lo:o + lo + 128],
                                    um_sb[:], rm_sb[:],
                                    start=False, stop=True,
                                    skip_group_check=True)
                        pt2 = ptp.tile([128, 2 * CH], F16, tag="pt",
                                       name=f"pt{hp}_{c}_{t}")
                        if s >= 0:
                            ia = ps2[:, 0:2 * CH].rearrange(
                                "p (two f) -> p two f", two=2)[:, :, lo:CH]
                            oa = pt2[:, 0:2 * CH].rearrange(
                                "p (two f) -> p two f", two=2)[:, :, lo:CH]
                            nc.scalar.activation(oa, ia, AF.Exp)
                        else:
                            nc.scalar.activation(pt2[:], ps2[:], AF.Exp)
                        for av, h, j in ((av0, h0, 0), (av1, h1, 1)):
                            nc.tensor.matmul(
                                av[:, lo:CH],
                                vsb[t][:, 65 * h:65 * h + 65],
                                pt2[:, CH * j + lo:CH * (j + 1)],
                                start=(t == 0), stop=(t == nkt - 1))
                    # drain the AV accumulators to SBUF so the psav banks
                    # recycle immediately; normalize runs from SBUF
                    halves = (c == NCH - 1)
                    for av, h in ((av0, h0), (av1, h1)):
                        dn = smsb.tile([1, CH], F32, tag="dn",
                                       name=f"dn{h}_{c}")
                        nc.vector.tensor_copy(dn[:], av[64:65, :])
                        av_sb = avs.tile([64, CH], F32, tag="avsb",
                                         name=f"avsb{h}_{c}")
                        nc.vector.tensor_copy(av_sb[:], av[0:64, :])
                        normalize(c, h, av_sb, dn, halves)

                def outproj(c):
                    for u in range(4 * c, 4 * c + 4):
                        o_sb = osb.tile([128, DM], F16, tag="osb",
                                        name=f"osb{u}")
                        for nchk in range(2):
                            ps = pvo.tile([128, CH], F32, tag="pvo",
                                          name=f"psout{u}_{nchk}")
                            for kk in range(2):
                                nc.tensor.matmul(
                                    ps[:],
                                    avt[kk][:, 128 * u:128 * (u + 1)],
                                    wot[kk][:, CH * nchk:CH * (nchk + 1)],
                                    start=(kk == 0), stop=(kk == 1))
                            # last chunk: scalar's exps are done, alternate
                            # the drain copies between Scalar and DVE
                            if c == NCH - 1 and (2 * u + nchk) % 2 == 1:
                                nc.scalar.activation(
                                    o_sb[:, CH * nchk:CH * (nchk + 1)],
                                    ps[:], AF.Copy)
                            else:
                                nc.vector.tensor_copy(
                                    o_sb[:, CH * nchk:CH * (nchk + 1)], ps[:])
                        nc.sync.dma_start(out[128 * u:128 * (u + 1), :],
                                          o_sb[:])

                vproj_rows(0)
                qkproj(0)
                for c in range(NCH):
                    attn_pair(c, 0)
                    if c + 1 < NCH:
                        vproj_rows(c + 1)
                    attn_pair(c, 1)
                    if c + 1 < NCH:
                        qkproj(c + 1)
                    outproj(c)

    nc.compile()
    return nc


def make_in_maps(x, w_qkv, w_out):
    """Per-core numpy input dicts. Core c = batch (c // 4) x head-group (c % 4)."""
    slopes = _slopes(NH)
    pos = np.arange(N, dtype=np.float32)
    kaug = np.stack([pos.astype(np.float16), np.ones(N, np.float16)])
    idx = np.arange(128)
    um = (idx[:, None] <= idx[None, :]).astype(np.float16)         # U[r,p]
    rmm = np.where(idx[:, None] > idx[None, :],
                   np.float16(MASK_NEG), np.float16(0.0))          # R[r,f]

    # column-blocked transposed x: [NCH*DM, CH], block b = xT[:, CH*b:CH*(b+1)]
    xtb16 = []
    for b in range(B):
        xT = np.ascontiguousarray(x[b].T).astype(np.float16)  # [DM, N]
        xtb16.append(np.ascontiguousarray(
            xT.reshape(DM, NCH, CH).transpose(1, 0, 2).reshape(NCH * DM, CH)))

    in_maps = []
    for c in range(8):
        b, g = c // 4, c % 4
        hg0 = H * g
        cols = slice(hg0 * D, (hg0 + H) * D)
        wqkv = np.concatenate([
            (w_qkv[:, cols.start:cols.stop] * SCALE).astype(np.float16),
            w_qkv[:, DM + cols.start:DM + cols.stop].astype(np.float16),
            w_qkv[:, 2 * DM + cols.start:2 * DM + cols.stop].astype(np.float16),
        ], axis=1)
        wo = w_out[cols, :].astype(np.float16)
        qa = np.empty((H, 2, N), np.float16)
        for h in range(H):
            s16 = np.float16(slopes[hg0 + h])
            qa[h, 0, :] = s16
            qa[h, 1, :] = (-np.float32(s16) * pos).astype(np.float16)
        in_maps.append({
            "xtb": xtb16[b], "wqkv": wqkv, "wo": wo,
            "qaug": qa, "kaug": kaug, "um": um, "rm": rmm,
        })
    return in_maps


_NC_CACHE = []


def _get_nc():
    if not _NC_CACHE:
        _NC_CACHE.append(build_program())
    return _NC_CACHE[0]


def run_cores(in_maps, **kw):
    nc = _get_nc()
    return bass_utils.run_bass_kernel_spmd(nc, in_maps, core_ids=list(range(8)), **kw)


def kernel(x, w_qkv, w_out, b_out):
    x = np.asarray(x, np.float32)
    w_qkv = np.asarray(w_qkv, np.float32)
    w_out = np.asarray(w_out, np.float32)
    b_out = np.asarray(b_out, np.float32)
    res = run_cores(make_in_maps(x, w_qkv, w_out))
    out = np.zeros((B, N, DM), np.float32)
    for c in range(8):
        out[c // 4] += res.results[c]["out"].astype(np.float32)
    out += b_out[None, None, :]
    return out
